# revision 11
# baseline (speedup 1.0000x reference)
"""Trainium2 Bass kernel for nn_Attention_78108275245493.

Dense cross+self attention block:
  h = LN_g1(x); q = (h Wq) * dh^-0.5 ; k,v = h Wkv ; + null kv token
  ck,cv = (flaxLN(context) Wc + bc) ;  attn over J = [self(2048) | null(1) | ctx(256)]
  out = LN_g2((softmax(q k^T) v) Wout)

Sharding: 8 cores = 2 batches x 4 sequence-quarters. Each core computes
k/v for its full batch (small duplicated work) and attention + output
projection for its own 512 query rows. No collectives. Inputs are
rotated per core so its query rows are always rows 0..511.

On-device layout is "transposed": h^T, k^T, q^T per head, so every
matmul contracts along partitions. attn@v uses v augmented with a ones
column so the softmax denominator falls out of the same matmul. Matmuls
run in float32r (full-rate fp32 mode on the PE).
"""

import sys

sys.path.insert(0, "/opt/trn_rl_repo")

import numpy as np

import concourse.bass as bass
import concourse.tile as tile
from concourse import bacc, mybir
from concourse.bass_utils import run_bass_kernel_spmd
from concourse.masks import make_identity

F32 = mybir.dt.float32
F32R = mybir.dt.float32r
AF = mybir.ActivationFunctionType
OP = mybir.AluOpType

B, N, DIM = 2, 2048, 1024
H, DH = 16, 64
CTX_N = 256
NCORES = 8
QPC = 512           # query rows per core
NT = N // 128       # 16 token tiles
CT = DIM // 128     # 8 contraction tiles
JT = 19             # padded key tiles: [self 16 | null+ctx 2.01 | pad]
JPAD = JT * 128     # 2432
JTOT = N + 1 + CTX_N  # 2305 real keys
HP = H // 2         # 8 head pairs

_CACHE = {}


def _bc_ap(src: bass.AP, nparts: int) -> bass.AP:
    """Broadcast a single-partition row [1, F] across nparts partitions."""
    ap = [[0, nparts]] + [list(a) for a in src.ap[1:]]
    return bass.AP(tensor=src.tensor, offset=src.offset, ap=ap)


def _emit(tc, t):
    nc = tc.nc
    ctxs = []

    def pool(name, bufs, space="SBUF"):
        p = tc.tile_pool(name=name, bufs=bufs, space=space)
        ctxs.append(p)
        return p.__enter__()

    const1 = pool("const1", 1)
    gvec = pool("gvec", 3)
    xpool = pool("xpool", 3)
    hpool = pool("hpool", 2)
    stat = pool("stat", 6)
    ppool = pool("ppool", 4)
    wqp = pool("wqp", 2)
    hts = pool("hts", 3)
    brec = pool("brec", 2)
    misc = pool("misc", 2)
    dram = pool("dram", 1, space="DRAM")
    psT = pool("psT", 2, space="PSUM")
    psM = pool("psM", 4, space="PSUM")
    psA = pool("psA", 2, space="PSUM")

    # ---- constants / persistent tiles ----
    ident = const1.tile([128, 128], F32, tag="ident")
    make_identity(nc, ident)
    eps_a = const1.tile([128, 1], F32, tag="eps_a")
    nc.vector.memset(eps_a, 1e-5)
    eps_c = const1.tile([128, 1], F32, tag="eps_c")
    nc.vector.memset(eps_c, 1e-6)

    g1b = gvec.tile([128, DIM], F32, tag="gv")
    nc.sync.dma_start(g1b, _bc_ap(t["g1"].ap()[None, :], 128))
    cgb = gvec.tile([128, DIM], F32, tag="gv")
    nc.sync.dma_start(cgb, _bc_ap(t["ctx_g"].ap()[None, :], 128))
    cbb = gvec.tile([128, DIM], F32, tag="gv2")
    nc.sync.dma_start(cbb, _bc_ap(t["ctx_b"].ap()[None, :], 128))

    wkv_sb = const1.tile([128, CT, 2 * DH], F32R, tag="wkv")
    nc.sync.dma_start(wkv_sb, t["Wkv"].ap().bitcast(F32R).rearrange("(o p) m -> p o m", p=128))
    wc_sb = const1.tile([128, CT, 2 * DH], F32R, tag="wc")
    nc.sync.dma_start(wc_sb, t["Wc"].ap().bitcast(F32R).rearrange("(o p) m -> p o m", p=128))
    bc_sb = const1.tile([128, 1], F32, tag="bc")
    nc.sync.dma_start(bc_sb, t["bc"].ap()[:, None])
    wout_sb = const1.tile([128, CT, DIM], F32R, tag="wout")
    for ct in range(CT):
        nc.sync.dma_start(wout_sb[:, ct, :],
                          t["Wout"].ap().bitcast(F32R)[ct * 128:(ct + 1) * 128, :])

    kT2 = const1.tile([128, JPAD], F32R, tag="kT2")
    v_aug = const1.tile([128, JT, DH + 2], F32R, tag="v_aug")  # [v | ones | pad]
    # f32r tiles cannot be memset directly; initialize from NEFF-embedded
    # constants. v_aug ones column marks valid keys: self tiles 0..15 all
    # rows, tiles 16/17 all rows (null + ctx 0..254), tile 18 row 0 only
    # (ctx 255); pads stay 0 so they contribute nothing to softmax.
    vinit = np.zeros((128, JT, DH + 2), np.float32)
    vinit[:, 0:18, DH] = 1.0
    vinit[0, 18, DH] = 1.0
    vinit_d = nc.inline_tensor(vinit, name="vinit")
    nc.sync.dma_start(v_aug, vinit_d.ap().bitcast(F32R))
    kpad_d = nc.inline_tensor(np.zeros((128, JPAD - JTOT), np.float32), name="kpad")
    nc.sync.dma_start(kT2[:, JTOT:], kpad_d.ap().bitcast(F32R))

    qT_sb = const1.tile([128, HP, QPC], F32R, tag="qT")
    aoT_sb = const1.tile([128, HP, QPC], F32R, tag="aoT")
    hTq_sb = const1.tile([128, CT, QPC], F32R, tag="hTq")
    chT_sb = const1.tile([128, CT, CTX_N], F32R, tag="chT")
    vT_sb = const1.tile([128, N], F32, tag="vT")

    hT_dram = dram.tile([CT, 128, N], F32R)

    def layernorm(x_t, eps_tile, width):
        """Returns (mean [128,1], rstd [128,1]) for tile [128, width]."""
        nsub = width // 512
        stats = stat.tile([128, nsub, 6], F32, tag="stats")
        for s in range(nsub):
            nc.vector.bn_stats(stats[:, s, :], x_t[:, s * 512:(s + 1) * 512])
        mv = stat.tile([128, 2], F32, tag="mv")
        nc.vector.bn_aggr(mv, stats)
        rstd = stat.tile([128, 1], F32, tag="rstd")
        nc.scalar.activation(rstd, mv[:, 1:2], AF.Sqrt, bias=eps_tile, scale=1.0)
        nc.vector.reciprocal(rstd, rstd)
        return mv, rstd

    # ---- phase A: h = LN(x) * g1, transposed to hT_dram ----
    for it in range(NT):
        x_t = xpool.tile([128, DIM], F32, tag="x")
        nc.sync.dma_start(x_t, t["xr"].ap()[it * 128:(it + 1) * 128, :])
        mv, rstd = layernorm(x_t, eps_a, DIM)
        h_t = hpool.tile([128, DIM], F32, tag="h")
        nc.vector.tensor_scalar(h_t, x_t, mv[:, 0:1], rstd, OP.subtract, OP.mult)
        nc.vector.tensor_mul(h_t, h_t, g1b)
        for ct in range(CT):
            tp = psT.tile([128, 128], F32, tag="pt")
            nc.tensor.transpose(tp, h_t[:, ct * 128:(ct + 1) * 128], ident)
            st = hts.tile([128, 128], F32R, tag="hstage")
            nc.vector.tensor_copy(out=st, in_=tp)
            nc.sync.dma_start(hT_dram[ct, :, it * 128:(it + 1) * 128], st)

    # hT columns 0..511 (this core's queries) stay resident for phase D
    for ct in range(CT):
        nc.sync.dma_start(hTq_sb[:, ct, :], hT_dram[ct, :, 0:QPC])

    # ---- phase B: k^T | v^T = Wkv^T @ h^T ----
    for nt in range(N // 512):
        ps = psM.tile([128, 512], F32, tag="mm")
        for ct in range(CT):
            r_t = hts.tile([128, 512], F32R, tag="hts")
            nc.sync.dma_start(r_t, hT_dram[ct, :, nt * 512:(nt + 1) * 512])
            nc.tensor.matmul(ps, wkv_sb[:, ct, :], r_t,
                             start=(ct == 0), stop=(ct == CT - 1))
        nc.vector.tensor_copy(out=kT2[0:64, nt * 512:(nt + 1) * 512], in_=ps[0:64, :])
        nc.vector.tensor_copy(out=vT_sb[64:128, nt * 512:(nt + 1) * 512], in_=ps[64:128, :])

    # v^T -> v rows of v_aug (PE transposes, identity sub-block at partitions 64:128)
    for tt in range(NT):
        tp = psT.tile([128, 128], F32, tag="pt")
        nc.tensor.transpose(tp[:, 0:64], vT_sb[64:128, tt * 128:(tt + 1) * 128],
                            ident[64:128, 64:128])
        nc.vector.tensor_copy(out=v_aug[:, tt, 0:DH], in_=tp[:, 0:64])

    # ---- phase C: context kv ----
    for tt in range(CTX_N // 128):
        c_t = xpool.tile([128, DIM], F32, tag="x")
        nc.sync.dma_start(c_t, t["context"].ap()[tt * 128:(tt + 1) * 128, :])
        mv, rstd = layernorm(c_t, eps_c, DIM)
        ch_t = hpool.tile([128, DIM], F32, tag="h")
        nc.vector.tensor_scalar(ch_t, c_t, mv[:, 0:1], rstd, OP.subtract, OP.mult)
        nc.vector.tensor_mul(ch_t, ch_t, cgb)
        nc.vector.tensor_add(ch_t, ch_t, cbb)
        for ct in range(CT):
            tp = psT.tile([128, 128], F32, tag="pt")
            nc.tensor.transpose(tp, ch_t[:, ct * 128:(ct + 1) * 128], ident)
            nc.vector.tensor_copy(out=chT_sb[:, ct, tt * 128:(tt + 1) * 128], in_=tp)

    psc = psM.tile([128, 512], F32, tag="mm")
    for ct in range(CT):
        nc.tensor.matmul(psc[:, 0:CTX_N], wc_sb[:, ct, :], chT_sb[:, ct, :],
                         start=(ct == 0), stop=(ct == CT - 1))
    # ck^T (+bc) into kT2 columns 2049..2304
    nc.vector.tensor_scalar(kT2[0:64, N + 1:N + 1 + CTX_N], psc[0:64, 0:CTX_N],
                            bc_sb[0:64], None, OP.add)
    cvT = misc.tile([128, CTX_N], F32, tag="cvT")
    nc.vector.tensor_scalar(cvT[64:128, :], psc[64:128, 0:CTX_N],
                            bc_sb[64:128], None, OP.add)
    cvs = misc.tile([128, 2, 64], F32R, tag="cvs")
    for tt in range(2):
        tp = psT.tile([128, 128], F32, tag="pt")
        nc.tensor.transpose(tp[:, 0:64], cvT[64:128, tt * 128:(tt + 1) * 128],
                            ident[64:128, 64:128])
        nc.vector.tensor_copy(out=cvs[:, tt, :], in_=tp[:, 0:64])
    # scatter ctx v rows (j = 2049..2304) into v_aug / v_aug_o; +1 partition shift
    for dst, off in ((v_aug, 0),):
        nc.sync.dma_start(dst[1:128, 16, off:off + 64], cvs[0:127, 0, :])
        nc.sync.dma_start(dst[0:1, 17, off:off + 64], cvs[127:128, 0, :])
        nc.sync.dma_start(dst[1:128, 17, off:off + 64], cvs[0:127, 1, :])
        nc.sync.dma_start(dst[0:1, 18, off:off + 64], cvs[127:128, 1, :])
        nc.sync.dma_start(dst[0:1, 16, off:off + 64], t["null_kv"].ap().bitcast(F32R)[1:2, :])
    # null k column (j = 2048)
    nc.sync.dma_start(kT2[0:64, N:N + 1], t["null_kv"].ap().bitcast(F32R)[0:1, :].rearrange("a d -> d a"))
    # duplicate k^T into partitions 64:128 for row-packed sim matmuls
    nc.sync.dma_start(kT2[64:128, :], kT2[0:64, :])

    # ---- phase D: q^T per head pair ----
    for hp in range(HP):
        wq_t = wqp.tile([128, CT, 128], F32R, tag="wq")
        nc.sync.dma_start(
            wq_t, t["Wq"].ap().bitcast(F32R)[:, hp * 128:(hp + 1) * 128].rearrange("(o p) m -> p o m", p=128))
        psq = psM.tile([128, 512], F32, tag="mm")
        for ct in range(CT):
            nc.tensor.matmul(psq, wq_t[:, ct, :], hTq_sb[:, ct, :],
                             start=(ct == 0), stop=(ct == CT - 1))
        nc.vector.tensor_copy(out=qT_sb[:, hp, :], in_=psq)

    # ---- phase E: attention, one head pair at a time ----
    scale = float(DH) ** -0.5
    for hp in range(HP):
        acc_e = psA.tile([128, 512], F32, tag="acc")
        acc_o = psA.tile([128, 512], F32, tag="acc")
        for jt in range(JT):
            js = slice(jt * 128, (jt + 1) * 128)
            ps_e = psM.tile([128, 512], F32, tag="mm")
            nc.tensor.matmul(ps_e, kT2[0:64, js], qT_sb[0:64, hp, :],
                             start=True, stop=True, tile_position=(0, 0))
            ps_o = psM.tile([128, 512], F32, tag="mm")
            nc.tensor.matmul(ps_o, kT2[64:128, js], qT_sb[64:128, hp, :],
                             start=True, stop=True, tile_position=(64, 0))
            p_e = ppool.tile([128, 512], F32R, tag="p")
            nc.scalar.activation(p_e, ps_e, AF.Exp, scale=scale)
            p_o = ppool.tile([128, 512], F32R, tag="p")
            nc.scalar.activation(p_o, ps_o, AF.Exp, scale=scale)
            nc.tensor.matmul(acc_e[0:DH + 2, :], v_aug[:, jt, :], p_e,
                             start=(jt == 0), stop=(jt == JT - 1),
                             skip_group_check=True)
            nc.tensor.matmul(acc_o[0:DH + 2, :], v_aug[:, jt, :], p_o,
                             start=(jt == 0), stop=(jt == JT - 1),
                             skip_group_check=True)
        rec_e = brec.tile([128, 512], F32, tag="rec")
        rec_o = brec.tile([128, 512], F32, tag="rec")
        nc.vector.reciprocal(rec_e[DH:DH + 1, :], acc_e[DH:DH + 1, :])
        nc.vector.reciprocal(rec_o[DH:DH + 1, :], acc_o[DH:DH + 1, :])
        # partition_broadcast reads partition 0 of its source; shift first
        nc.sync.dma_start(rec_e[0:1, :], rec_e[DH:DH + 1, :])
        nc.sync.dma_start(rec_o[0:1, :], rec_o[DH:DH + 1, :])
        br_e = brec.tile([128, 512], F32, tag="br")
        br_o = brec.tile([128, 512], F32, tag="br")
        nc.gpsimd.partition_broadcast(br_e[0:64, :], rec_e[0:1, :], channels=64)
        nc.gpsimd.partition_broadcast(br_o[0:64, :], rec_o[0:1, :], channels=64)
        nc.vector.tensor_mul(aoT_sb[0:64, hp, :], acc_e[0:64, :], br_e[0:64, :])
        tmp_o = brec.tile([128, 512], F32R, tag="tmp")
        nc.vector.tensor_mul(tmp_o[0:64, :], acc_o[0:64, :], br_o[0:64, :])
        nc.sync.dma_start(aoT_sb[64:128, hp, :], tmp_o[0:64, :])

    # ---- phase F: y = LN((attn_out) @ Wout) * g2 ----
    g2b = gvec.tile([128, DIM], F32, tag="gv")
    nc.sync.dma_start(g2b, _bc_ap(t["g2"].ap()[None, :], 128))
    for it in range(QPC // 128):
        ps0 = psM.tile([128, 512], F32, tag="mm")
        ps1 = psM.tile([128, 512], F32, tag="mm")
        isl = slice(it * 128, (it + 1) * 128)
        for ct in range(CT):
            nc.tensor.matmul(ps0, aoT_sb[:, ct, isl], wout_sb[:, ct, 0:512],
                             start=(ct == 0), stop=(ct == CT - 1), skip_group_check=True)
            nc.tensor.matmul(ps1, aoT_sb[:, ct, isl], wout_sb[:, ct, 512:1024],
                             start=(ct == 0), stop=(ct == CT - 1), skip_group_check=True)
        stats = stat.tile([128, 2, 6], F32, tag="stats")
        nc.vector.bn_stats(stats[:, 0, :], ps0)
        nc.vector.bn_stats(stats[:, 1, :], ps1)
        mv = stat.tile([128, 2], F32, tag="mv")
        nc.vector.bn_aggr(mv, stats)
        rstd = stat.tile([128, 1], F32, tag="rstd")
        nc.scalar.activation(rstd, mv[:, 1:2], AF.Sqrt, bias=eps_a, scale=1.0)
        nc.vector.reciprocal(rstd, rstd)
        y_t = xpool.tile([128, DIM], F32, tag="x")
        nc.vector.tensor_scalar(y_t[:, 0:512], ps0, mv[:, 0:1], rstd, OP.subtract, OP.mult)
        nc.vector.tensor_scalar(y_t[:, 512:1024], ps1, mv[:, 0:1], rstd, OP.subtract, OP.mult)
        nc.vector.tensor_mul(y_t, y_t, g2b)
        nc.sync.dma_start(t["y"].ap()[isl, :], y_t)

    for p in reversed(ctxs):
        p.__exit__(None, None, None)


def build():
    if "nc" in _CACHE:
        return _CACHE["nc"]
    nc = bacc.Bacc("TRN2", target_bir_lowering=False, debug=False, num_devices=NCORES)
    t = {
        "xr": nc.dram_tensor("xr", [N, DIM], F32, kind="ExternalInput"),
        "context": nc.dram_tensor("context", [CTX_N, DIM], F32, kind="ExternalInput"),
        "g1": nc.dram_tensor("g1", [DIM], F32, kind="ExternalInput"),
        "g2": nc.dram_tensor("g2", [DIM], F32, kind="ExternalInput"),
        "ctx_g": nc.dram_tensor("ctx_g", [DIM], F32, kind="ExternalInput"),
        "ctx_b": nc.dram_tensor("ctx_b", [DIM], F32, kind="ExternalInput"),
        "Wq": nc.dram_tensor("Wq", [DIM, H * DH], F32, kind="ExternalInput"),
        "Wkv": nc.dram_tensor("Wkv", [DIM, 2 * DH], F32, kind="ExternalInput"),
        "Wc": nc.dram_tensor("Wc", [DIM, 2 * DH], F32, kind="ExternalInput"),
        "bc": nc.dram_tensor("bc", [2 * DH], F32, kind="ExternalInput"),
        "Wout": nc.dram_tensor("Wout", [H * DH, DIM], F32, kind="ExternalInput"),
        "null_kv": nc.dram_tensor("null_kv", [2, DH], F32, kind="ExternalInput"),
        "y": nc.dram_tensor("y", [QPC, DIM], F32, kind="ExternalOutput"),
    }
    with tile.TileContext(nc) as tc:
        _emit(tc, t)
    nc.compile()
    _CACHE["nc"] = nc
    return nc


def shard_inputs(inputs) -> list[dict[str, np.ndarray]]:
    f32 = lambda a: np.ascontiguousarray(np.asarray(a, dtype=np.float32))
    x = f32(inputs["x"])
    ctx = f32(inputs["context"])
    shared = {
        "g1": f32(inputs["g1"]), "g2": f32(inputs["g2"]),
        "ctx_g": f32(inputs["ctx_g"]), "ctx_b": f32(inputs["ctx_b"]),
        "Wq": f32(inputs["Wq"]), "Wkv": f32(inputs["Wkv"]),
        "Wc": f32(inputs["Wc"]), "bc": f32(inputs["bc"]),
        "Wout": f32(inputs["Wout"]), "null_kv": f32(inputs["null_kv"]),
    }
    in_maps = []
    for core in range(NCORES):
        b, r = divmod(core, NCORES // B)
        xb = x[b]
        xr = np.ascontiguousarray(np.concatenate([xb[r * QPC:], xb[:r * QPC]], axis=0))
        in_maps.append({"xr": xr, "context": ctx[b], **shared})
    return in_maps


def gather_outputs(results) -> np.ndarray:
    y = np.empty((B, N, DIM), np.float32)
    for core in range(NCORES):
        b, r = divmod(core, NCORES // B)
        y[b, r * QPC:(r + 1) * QPC] = results[core]["y"]
    return y


def kernel(**inputs) -> np.ndarray:
    nc = build()
    res = run_bass_kernel_spmd(nc, shard_inputs(inputs), list(range(NCORES)))
    return gather_outputs(res.results)


# revision 21
# speedup vs baseline: 5566.5863x; 5566.5863x over previous
"""Trainium2 Bass kernel for nn_Attention_78108275245493.

Dense cross+self attention block:
  h = LN_g1(x); q = (h Wq) * dh^-0.5 ; k,v = h Wkv ; + null kv token
  ck,cv = (flaxLN(context) Wc + bc) ;  attn over J = [self(2048) | null(1) | ctx(256)]
  out = LN_g2((softmax(q k^T) v) Wout)

Sharding: 8 cores = 2 batches x 4 sequence-quarters. Each core computes
k/v for its full batch (small duplicated work) and attention + output
projection for its own 512 query rows. No collectives. Inputs are
rotated per core so its query rows are always rows 0..511.

Host-side prep folds the LN scales into the projection weights
(Wq' = diag(g1) Wq, Wkv' = diag(g1) Wkv, Wc' = diag(ctx_g) Wc,
bc' = bc + ctx_b @ Wc), so the device only computes plain layernorms.

On-device layout is "transposed": h^T, k^T, q^T per head, so every
matmul contracts along partitions. h^T is produced in four [1024, 512]
SBUF window slabs that are consumed immediately by the k/v and q
projections (no HBM round trip). attn@v uses v augmented with a ones
column so the softmax denominator falls out of the same matmul.
Matmuls run in float32r (full-rate fp32 mode on the PE).
"""

import sys

sys.path.insert(0, "/opt/trn_rl_repo")

import numpy as np

import concourse.bass as bass
import concourse.tile as tile
from concourse import bacc, mybir
from concourse.bass_utils import run_bass_kernel_spmd
from concourse.masks import make_identity

F32 = mybir.dt.float32
F32R = mybir.dt.float32r
AF = mybir.ActivationFunctionType
OP = mybir.AluOpType

B, N, DIM = 2, 2048, 1024
H, DH = 16, 64
CTX_N = 256
NCORES = 8
QPC = 512           # query rows per core
NT = N // 128       # 16 token tiles
CT = DIM // 128     # 8 contraction tiles
JT = 19             # padded key tiles: [self 16 | null+ctx 2.01 | pad]
JPAD = JT * 128     # 2432
JTOT = N + 1 + CTX_N  # 2305 real keys
HP = H // 2         # 8 head pairs
NW = N // 512       # 4 h^T window slabs

REPEAT = 1          # >1 wraps the body in a hardware loop (timing runs only)

_CACHE = {}


def _bc_ap(src: bass.AP, nparts: int) -> bass.AP:
    """Broadcast a single-partition row [1, F] across nparts partitions."""
    ap = [[0, nparts]] + [list(a) for a in src.ap[1:]]
    return bass.AP(tensor=src.tensor, offset=src.offset, ap=ap)


def _emit(tc, t):
    nc = tc.nc
    ctxs = []

    def pool(name, bufs, space="SBUF"):
        p = tc.tile_pool(name=name, bufs=bufs, space=space)
        ctxs.append(p)
        return p.__enter__()

    const1 = pool("const1", 1)
    gvec = pool("gvec", 1)
    xpool = pool("xpool", 8)
    stat = pool("stat", 6)
    ppool = pool("ppool", 6)
    wqp = pool("wqp", 2)
    brec = pool("brec", 2)
    misc = pool("misc", 2)
    winp = pool("winp", 2)
    vtp = pool("vtp", 2)
    psM = pool("psM", 3, space="PSUM")
    psA = pool("psA", 2, space="PSUM")

    # ---- constants / persistent tiles ----
    ident = const1.tile([128, 128], F32, tag="ident")
    make_identity(nc, ident)
    eps_a = const1.tile([128, 1], F32, tag="eps_a")
    nc.vector.memset(eps_a, 1e-5)
    eps_c = const1.tile([128, 1], F32, tag="eps_c")
    nc.vector.memset(eps_c, 1e-6)

    wkv_sb = const1.tile([128, CT, 2 * DH], F32R, tag="wkv")
    nc.sync.dma_start(wkv_sb, t["Wkv"].ap().bitcast(F32R).rearrange("(o p) m -> p o m", p=128))
    wc_sb = const1.tile([128, CT, 2 * DH], F32R, tag="wc")
    nc.sync.dma_start(wc_sb, t["Wc"].ap().bitcast(F32R).rearrange("(o p) m -> p o m", p=128))
    bc_sb = const1.tile([128, 1], F32, tag="bc")
    nc.sync.dma_start(bc_sb, t["bc"].ap()[:, None])

    kT2 = const1.tile([128, JPAD], F32R, tag="kT2")
    v_aug = const1.tile([128, JT, DH + 2], F32R, tag="v_aug")  # [v | ones | pad]
    # f32r tiles cannot be memset directly; initialize from NEFF-embedded
    # constants. v_aug ones column marks valid keys: self tiles 0..15 all
    # rows, tiles 16/17 all rows (null + ctx 0..254), tile 18 row 0 only
    # (ctx 255); pads stay 0 so they contribute nothing to softmax.
    vinit = np.zeros((128, JT, DH + 2), np.float32)
    vinit[:, 0:18, DH] = 1.0
    vinit[0, 18, DH] = 1.0
    vinit_d = nc.inline_tensor(vinit, name="vinit")
    nc.sync.dma_start(v_aug, vinit_d.ap().bitcast(F32R))
    kpad_d = nc.inline_tensor(np.zeros((128, JPAD - JTOT), np.float32), name="kpad")
    nc.sync.dma_start(kT2[:, JTOT:], kpad_d.ap().bitcast(F32R))

    qT_sb = const1.tile([128, HP, QPC], F32R, tag="qT")
    aoT_sb = const1.tile([128, HP, QPC], F32R, tag="aoT")

    rep_ctx = tc.For_i(0, REPEAT, 1) if REPEAT > 1 else None
    if rep_ctx is not None:
        rep_ctx.__enter__()

    def layernorm(x_t, eps_tile, width):
        """In-place layernorm (no scale) of tile [128, width]."""
        nsub = width // 512
        stats = stat.tile([128, nsub, 6], F32, tag="stats")
        for s in range(nsub):
            nc.vector.bn_stats(stats[:, s, :], x_t[:, s * 512:(s + 1) * 512])
        mv = stat.tile([128, 2], F32, tag="mv")
        nc.vector.bn_aggr(mv, stats)
        rstd = stat.tile([128, 1], F32, tag="rstd")
        nc.scalar.activation(rstd, mv[:, 1:2], AF.Sqrt, bias=eps_tile, scale=1.0)
        nc.vector.reciprocal(rstd, rstd)
        nc.vector.tensor_scalar(x_t, x_t, mv[:, 0:1], rstd, OP.subtract, OP.mult)

    # ---- phase C: context kv ----
    chT_sb = winp.tile([128, CT, 512], F32R, tag="win")
    cts = []
    for tt in range(CTX_N // 128):
        c_t = xpool.tile([128, DIM], F32, tag="x")
        nc.sync.dma_start(c_t, t["context"].ap()[tt * 128:(tt + 1) * 128, :])
        layernorm(c_t, eps_c, DIM)
        cts.append(c_t)
    for ct in range(CT):
        tp = psM.tile([128, 1024], F32, tag="mm")
        for tt in range(2):
            nc.tensor.transpose(tp[:, tt * 128:(tt + 1) * 128],
                                cts[tt][:, ct * 128:(ct + 1) * 128], ident)
        if ct % 2 == 0:
            nc.vector.tensor_copy(out=chT_sb[:, ct, 0:256], in_=tp[:, 0:256])
        else:
            nc.scalar.copy(out=chT_sb[:, ct, 0:256], in_=tp[:, 0:256])

    psc = psM.tile([128, 1024], F32, tag="mm")
    for ct in range(CT):
        nc.tensor.matmul(psc[:, 0:CTX_N], wc_sb[:, ct, :], chT_sb[:, ct, 0:256],
                         start=(ct == 0), stop=(ct == CT - 1))
    # ck^T (+bc) into kT2 columns 2049..2304
    nc.vector.tensor_scalar(kT2[0:64, N + 1:N + 1 + CTX_N], psc[0:64, 0:CTX_N],
                            bc_sb[0:64], None, OP.add)
    cvT = misc.tile([128, CTX_N], F32, tag="cvT")
    nc.vector.tensor_scalar(cvT[64:128, :], psc[64:128, 0:CTX_N],
                            bc_sb[64:128], None, OP.add)
    cvs = misc.tile([128, 2, 64], F32R, tag="cvs")
    tpc = psM.tile([128, 1024], F32, tag="mm")
    for tt in range(2):
        nc.tensor.transpose(tpc[:, tt * 64:(tt + 1) * 64],
                            cvT[64:128, tt * 128:(tt + 1) * 128],
                            ident[64:128, 64:128])
    nc.vector.tensor_copy(out=cvs[:, :, :], in_=tpc[:, 0:128].rearrange("p (a b) -> p a b", a=2))
    # scatter ctx v rows (j = 2049..2304) into v_aug; +1 partition shift
    nc.sync.dma_start(v_aug[1:128, 16, 0:64], cvs[0:127, 0, :])
    nc.sync.dma_start(v_aug[0:1, 17, 0:64], cvs[127:128, 0, :])
    nc.sync.dma_start(v_aug[1:128, 17, 0:64], cvs[0:127, 1, :])
    nc.sync.dma_start(v_aug[0:1, 18, 0:64], cvs[127:128, 1, :])
    nc.sync.dma_start(v_aug[0:1, 16, 0:64], t["null_kv"].ap().bitcast(F32R)[1:2, :])
    # null k column (j = 2048)
    nc.sync.dma_start(kT2[0:64, N:N + 1],
                      t["null_kv"].ap().bitcast(F32R)[0:1, :].rearrange("a d -> d a"))

    # ---- phase A/B/D fused: per 512-token window, h^T slab -> k/v (+q) ----
    for w in range(NW):
        win = winp.tile([128, CT, 512], F32R, tag="win")
        xts = []
        for i4 in range(4):
            it = w * 4 + i4
            x_t = xpool.tile([128, DIM], F32, tag="x")
            nc.sync.dma_start(x_t, t["xr"].ap()[it * 128:(it + 1) * 128, :])
            layernorm(x_t, eps_a, DIM)
            xts.append(x_t)
        # transposes batched per ct: 4 PE transposes share one psum tile,
        # drained by a single [128, 512] DVE copy into the window slab
        for ct in range(CT):
            tp = psM.tile([128, 1024], F32, tag="mm")
            for i4 in range(4):
                nc.tensor.transpose(tp[:, i4 * 128:(i4 + 1) * 128],
                                    xts[i4][:, ct * 128:(ct + 1) * 128], ident)
            if ct % 2 == 0:
                nc.vector.tensor_copy(out=win[:, ct, :], in_=tp[:, 0:512])
            else:
                nc.scalar.copy(out=win[:, ct, :], in_=tp[:, 0:512])
        # k^T | v^T columns for this window
        psk = psM.tile([128, 1024], F32, tag="mm")
        for ct in range(CT):
            nc.tensor.matmul(psk[:, 0:512], wkv_sb[:, ct, :], win[:, ct, :],
                             start=(ct == 0), stop=(ct == CT - 1))
        nc.scalar.copy(out=kT2[0:64, w * 512:(w + 1) * 512], in_=psk[0:64, 0:512])
        vt = vtp.tile([128, 512], F32, tag="vt")
        nc.vector.tensor_copy(out=vt[64:128, :], in_=psk[64:128, 0:512])
        tpv = psM.tile([128, 1024], F32, tag="mm")
        for k4 in range(4):
            nc.tensor.transpose(tpv[:, k4 * 64:(k4 + 1) * 64],
                                vt[64:128, k4 * 128:(k4 + 1) * 128],
                                ident[64:128, 64:128])
        nc.vector.tensor_copy(out=v_aug[:, w * 4:(w + 1) * 4, 0:DH],
                              in_=tpv[:, 0:256].rearrange("p (a b) -> p a b", a=4))
        if w == 0:
            # q^T for all head pairs from the first window (this core's queries)
            for hp in range(HP):
                wq_t = wqp.tile([128, CT, 128], F32R, tag="wq")
                nc.sync.dma_start(
                    wq_t, t["Wq"].ap().bitcast(F32R)[:, hp * 128:(hp + 1) * 128]
                    .rearrange("(o p) m -> p o m", p=128))
                psq = psM.tile([128, 1024], F32, tag="mm")
                for ct in range(CT):
                    nc.tensor.matmul(psq[:, 0:512], wq_t[:, ct, :], win[:, ct, :],
                                     start=(ct == 0), stop=(ct == CT - 1))
                if hp % 2 == 0:
                    nc.vector.tensor_copy(out=qT_sb[:, hp, :], in_=psq[:, 0:512])
                else:
                    nc.scalar.copy(out=qT_sb[:, hp, :], in_=psq[:, 0:512])

    # duplicate k^T into partitions 64:128 for row-packed sim matmuls
    nc.sync.dma_start(kT2[64:128, :], kT2[0:64, :])

    wout_sb = const1.tile([128, CT, DIM], F32R, tag="wout")
    for ct in range(CT):
        nc.sync.dma_start(wout_sb[:, ct, :],
                          t["Wout"].ap().bitcast(F32R)[ct * 128:(ct + 1) * 128, :])

    # ---- phase E: attention, one head pair at a time ----
    scale = float(DH) ** -0.5
    def pair_tail(acc_e, acc_o, hp):
        def emit():
            rec_e = brec.tile([128, 512], F32, tag="rec")
            rec_o = brec.tile([128, 512], F32, tag="rec")
            nc.vector.reciprocal(rec_e[DH:DH + 1, :], acc_e[DH:DH + 1, :])
            nc.vector.reciprocal(rec_o[DH:DH + 1, :], acc_o[DH:DH + 1, :])
            # partition_broadcast reads partition 0 of its source; shift first
            nc.sync.dma_start(rec_e[0:1, :], rec_e[DH:DH + 1, :])
            nc.sync.dma_start(rec_o[0:1, :], rec_o[DH:DH + 1, :])
            br_e = brec.tile([128, 512], F32, tag="br")
            br_o = brec.tile([128, 512], F32, tag="br")
            nc.gpsimd.partition_broadcast(br_e[0:64, :], rec_e[0:1, :], channels=64)
            nc.gpsimd.partition_broadcast(br_o[0:64, :], rec_o[0:1, :], channels=64)
            nc.vector.tensor_mul(aoT_sb[0:64, hp, :], acc_e[0:64, :], br_e[0:64, :])
            tmp_o = brec.tile([128, 512], F32R, tag="tmp")
            nc.vector.tensor_mul(tmp_o[0:64, :], acc_o[0:64, :], br_o[0:64, :])
            nc.sync.dma_start(aoT_sb[64:128, hp, :], tmp_o[0:64, :])
        return emit

    tail = None
    for hp in range(HP):
        acc_e = psA.tile([128, 512], F32, tag="acc")
        acc_o = psA.tile([128, 512], F32, tag="acc")
        # software pipeline: emit sim/exp for jt before attn@v of jt-1 so the
        # PE always has independent sim work while ACT computes the exps
        pending = None
        for jt in range(JT):
            js = slice(jt * 128, (jt + 1) * 128)
            ps = psM.tile([128, 1024], F32, tag="mm")
            nc.tensor.matmul(ps[:, 0:512], kT2[0:64, js], qT_sb[0:64, hp, :],
                             start=True, stop=True, tile_position=(0, 0),
                             skip_group_check=True)
            nc.tensor.matmul(ps[:, 512:1024], kT2[64:128, js], qT_sb[64:128, hp, :],
                             start=True, stop=True, tile_position=(64, 0),
                             skip_group_check=True)
            p_t = ppool.tile([128, 1024], F32R, tag="p")
            nc.scalar.activation(p_t, ps, AF.Exp, scale=scale)
            if jt == 2 and tail is not None:
                # emit the previous pair's normalize here so it overlaps this
                # pair's sims instead of stalling the ACT pipeline
                tail()
                tail = None
            if pending is not None:
                _pt, _jt = pending
                nc.tensor.matmul(acc_e[0:DH + 2, :], v_aug[:, _jt, :], _pt[:, 0:512],
                                 start=(_jt == 0), stop=False,
                                 skip_group_check=True)
                nc.tensor.matmul(acc_o[0:DH + 2, :], v_aug[:, _jt, :], _pt[:, 512:1024],
                                 start=(_jt == 0), stop=False,
                                 skip_group_check=True)
            pending = (p_t, jt)
        _pt, _jt = pending
        nc.tensor.matmul(acc_e[0:DH + 2, :], v_aug[:, _jt, :], _pt[:, 0:512],
                         start=False, stop=True, skip_group_check=True)
        nc.tensor.matmul(acc_o[0:DH + 2, :], v_aug[:, _jt, :], _pt[:, 512:1024],
                         start=False, stop=True, skip_group_check=True)
        tail = pair_tail(acc_e, acc_o, hp)
    tail()

    # ---- phase F: y = LN(y_acc) * g2 ----
    g2b = gvec.tile([128, DIM], F32, tag="gv")
    nc.sync.dma_start(g2b, _bc_ap(t["g2"].ap()[None, :], 128))
    for it in range(QPC // 128):
        psy = psM.tile([128, 1024], F32, tag="mm")
        isl = slice(it * 128, (it + 1) * 128)
        for ct in range(CT):
            nc.tensor.matmul(psy[:, 0:512], aoT_sb[:, ct, isl], wout_sb[:, ct, 0:512],
                             start=(ct == 0), stop=(ct == CT - 1), skip_group_check=True)
            nc.tensor.matmul(psy[:, 512:1024], aoT_sb[:, ct, isl], wout_sb[:, ct, 512:1024],
                             start=(ct == 0), stop=(ct == CT - 1), skip_group_check=True)
        stats = stat.tile([128, 2, 6], F32, tag="stats")
        nc.vector.bn_stats(stats[:, 0, :], psy[:, 0:512])
        nc.vector.bn_stats(stats[:, 1, :], psy[:, 512:1024])
        mv = stat.tile([128, 2], F32, tag="mv")
        nc.vector.bn_aggr(mv, stats)
        rstd = stat.tile([128, 1], F32, tag="rstd")
        nc.scalar.activation(rstd, mv[:, 1:2], AF.Sqrt, bias=eps_a, scale=1.0)
        nc.vector.reciprocal(rstd, rstd)
        y_t = xpool.tile([128, DIM], F32, tag="x")
        nc.vector.tensor_scalar(y_t, psy, mv[:, 0:1], rstd, OP.subtract, OP.mult)
        nc.vector.tensor_mul(y_t, y_t, g2b)
        nc.sync.dma_start(t["y"].ap()[isl, :], y_t)

    if rep_ctx is not None:
        rep_ctx.__exit__(None, None, None)

    for p in reversed(ctxs):
        p.__exit__(None, None, None)


def build():
    if ("nc", REPEAT) in _CACHE:
        return _CACHE[("nc", REPEAT)]
    nc = bacc.Bacc("TRN2", target_bir_lowering=False, debug=False, num_devices=NCORES)
    t = {
        "xr": nc.dram_tensor("xr", [N, DIM], F32, kind="ExternalInput"),
        "context": nc.dram_tensor("context", [CTX_N, DIM], F32, kind="ExternalInput"),
        "g2": nc.dram_tensor("g2", [DIM], F32, kind="ExternalInput"),
        "Wq": nc.dram_tensor("Wq", [DIM, H * DH], F32, kind="ExternalInput"),
        "Wkv": nc.dram_tensor("Wkv", [DIM, 2 * DH], F32, kind="ExternalInput"),
        "Wc": nc.dram_tensor("Wc", [DIM, 2 * DH], F32, kind="ExternalInput"),
        "bc": nc.dram_tensor("bc", [2 * DH], F32, kind="ExternalInput"),
        "Wout": nc.dram_tensor("Wout", [H * DH, DIM], F32, kind="ExternalInput"),
        "null_kv": nc.dram_tensor("null_kv", [2, DH], F32, kind="ExternalInput"),
        "y": nc.dram_tensor("y", [QPC, DIM], F32, kind="ExternalOutput"),
    }
    with tile.TileContext(nc) as tc:
        _emit(tc, t)
    nc.compile()
    _CACHE[("nc", REPEAT)] = nc
    return nc


def shard_inputs(inputs) -> list[dict[str, np.ndarray]]:
    f32 = lambda a: np.ascontiguousarray(np.asarray(a, dtype=np.float32))
    x = f32(inputs["x"])
    ctx = f32(inputs["context"])
    # fold LN scales/bias into the projection weights (exact algebra:
    # LN0 = (x-m)/s, h = LN0*g1, h @ W == LN0 @ (diag(g1) W))
    g1 = f32(inputs["g1"])[:, None]
    ctx_g = f32(inputs["ctx_g"])[:, None]
    ctx_b = f32(inputs["ctx_b"])
    Wc = f32(inputs["Wc"])
    shared = {
        "g2": f32(inputs["g2"]),
        "Wq": np.ascontiguousarray(g1 * f32(inputs["Wq"])),
        "Wkv": np.ascontiguousarray(g1 * f32(inputs["Wkv"])),
        "Wc": np.ascontiguousarray(ctx_g * Wc),
        "bc": np.ascontiguousarray(f32(inputs["bc"]) + ctx_b @ Wc),
        "Wout": f32(inputs["Wout"]),
        "null_kv": f32(inputs["null_kv"]),
    }
    in_maps = []
    for core in range(NCORES):
        b, r = divmod(core, NCORES // B)
        xb = x[b]
        xr = np.ascontiguousarray(np.concatenate([xb[r * QPC:], xb[:r * QPC]], axis=0))
        in_maps.append({"xr": xr, "context": ctx[b], **shared})
    return in_maps


def gather_outputs(results) -> np.ndarray:
    y = np.empty((B, N, DIM), np.float32)
    for core in range(NCORES):
        b, r = divmod(core, NCORES // B)
        y[b, r * QPC:(r + 1) * QPC] = results[core]["y"]
    return y


def kernel(**inputs) -> np.ndarray:
    nc = build()
    res = run_bass_kernel_spmd(nc, shard_inputs(inputs), list(range(NCORES)))
    return gather_outputs(res.results)
